# revision 1
# baseline (speedup 1.0000x reference)
"""Bass/Trainium2 kernel for nn_GCNNTemporal (GNN message passing over object masks).

Reference computation (B=4 samples, O=8 objects, C=256, HID=128, H=W=64):
  states = relu(conv3x3(concat(feats, mask_o)))          per (sample, object)
  2x:  states_o = relu(conv3x3(concat(states_o, sum_{j!=o} states_j)))
  out_o = sigmoid(conv3x3(concat(feats, states_o)))

Sharding: 2 cores per sample, 4 objects per core. The neighbor sum needs
sum over all 8 objects -> pairwise AllReduce of the local 4-object sum.

Conv3x3 SAME is 9 shifted matmuls accumulating in PSUM over a zero-padded
flat [128, 66*66] layout (shifts are pure AP offsets). Algebra:
  - enc: conv(feats) shared by the 4 objects; per-object mask contribution
    via a single K=9 im2col matmul per chunk.
  - gcn: agg_o = total - states_o  =>  out_o = relu(conv(states_o, w1-w2)
    + conv(total, w2) + b); conv(total, w2) computed once per core.
  - readout (Cout=1): 4 objects stacked into K with block-diagonal weights
    (M=4); the shared feats part folded in by replicating its weight column.
Matmuls run in float32r (1 cycle/row for N>=256) with fp32 PSUM accumulation.
"""
import sys
sys.path.insert(0, '/opt/trn_rl_repo')
import numpy as np

B, O, C, HID, H, W = 4, 8, 256, 128, 64, 64
STEPS = 2
N_CORES = 8

Wp = W + 2                 # padded row width
PADF = (H + 2) * Wp        # 4356 padded flat size
EXT = PADF + 2             # 4358 with +-1 guard elements (data at offset 1)
NINT = H * Wp              # 4224 matmul output columns per image
INT0 = 1 + Wp              # EXT offset of interior (row 1, col 0) = 67
CHUNKS = [(i * 480, 480) for i in range(8)] + [(3840, 384)]
TAPS = [(ky, kx) for ky in range(3) for kx in range(3)]

_PROG_CACHE = {}


def _build_program(repeat=1):
    import concourse.tile as tile
    from concourse import bacc, mybir

    AF = mybir.ActivationFunctionType
    F32 = mybir.dt.float32
    F16 = mybir.dt.float16

    nc = bacc.Bacc("TRN2", target_bir_lowering=False, debug=False,
                   num_devices=N_CORES)

    # ---- DRAM I/O (all operands fp16; biases + collective payload fp32) ----
    feats_ap = nc.dram_tensor("feats", [2, 128, EXT], F16, kind="ExternalInput").ap()
    mcols_ap = nc.dram_tensor("mcols", [4, 9, NINT], F16, kind="ExternalInput").ap()
    encw_ap = nc.dram_tensor("encw", [128, 2 * 9 * 128], F16, kind="ExternalInput").ap()
    maskw_ap = nc.dram_tensor("maskw", [128, 128], F16, kind="ExternalInput").ap()
    gcnw12_ap = nc.dram_tensor("gcnw12", [128, 9 * 128], F16, kind="ExternalInput").ap()
    gcnw2_ap = nc.dram_tensor("gcnw2", [128, 9 * 128], F16, kind="ExternalInput").ap()
    row_ap = nc.dram_tensor("row", [128, 6 * 9 * 4], F16, kind="ExternalInput").ap()
    ident_ap = nc.dram_tensor("ident", [128, 128], F16, kind="ExternalInput").ap()
    encb_ap = nc.dram_tensor("encb", [128, 1], F32, kind="ExternalInput").ap()
    gcnb_ap = nc.dram_tensor("gcnb", [128, 1], F32, kind="ExternalInput").ap()
    rob_ap = nc.dram_tensor("rob", [4, 1], F32, kind="ExternalInput").ap()
    out_ap = nc.dram_tensor("out", [4, H * W], F32, kind="ExternalOutput").ap()

    with tile.TileContext(nc) as tc:
        with tc.tile_pool(name="persist", bufs=1) as pp, \
             tc.tile_pool(name="psum", bufs=8, space="PSUM") as psp, \
             tc.tile_pool(name="dram", bufs=1, space="DRAM") as dp:

            # ---- persistent SBUF ----
            sts = [pp.tile([128, EXT], F16, tag=f"st{i}", name=f"st{i}")
                   for i in range(5)]
            feats_sb = [pp.tile([128, EXT], F16, tag=f"feat{k}", name=f"feat{k}")
                        for k in range(2)]
            shared_sb = pp.tile([128, NINT], F16, tag="shared")   # encF / gcnT
            gcnw12_sb = pp.tile([128, 9 * 128], F16, tag="gw12")
            gcnw2_sb = pp.tile([128, 9 * 128], F16, tag="gw2")
            row_sb = pp.tile([128, 6 * 9 * 4], F16, tag="row")
            encb_sb = pp.tile([128, 1], F32, tag="encb")
            gcnb_sb = pp.tile([128, 1], F32, tag="gcnb")
            rob_sb = pp.tile([4, 1], F32, tag="rob")
            lsum_sb = pp.tile([128, NINT], F32, tag="lsum")
            ident_sb = pp.tile([128, 128], F16, tag="ident")

            # tiny warm-up collective absorbs CC-stream init latency
            ccw_in = dp.tile([1, 1], F32, tag="ccwin")
            ccw_out = dp.tile([1, 1], F32, tag="ccwout")
            warm_sb = pp.tile([1, 1], F32, tag="warm")
            nc.vector.memset(warm_sb[:], 0.0)
            nc.sync.dma_start(out=ccw_in[:], in_=warm_sb[:])
            nc.gpsimd.collective_compute(
                "AllReduce", mybir.AluOpType.add,
                replica_groups=[[0, 1], [2, 3], [4, 5], [6, 7]],
                ins=[ccw_in.opt()], outs=[ccw_out.opt()])

            # zero guard/border regions once
            for t_ in sts:
                nc.vector.memset(t_[:, 0:INT0], 0.0)
                nc.vector.memset(t_[:, INT0 + NINT:EXT], 0.0)

            def conv_mms(ps, w_sb, w_idx, src, nb, nw, first, last, m=128):
                """9 accumulating tap matmuls into psum ps."""
                for t, (ky, kx) in enumerate(TAPS):
                    off = ky * Wp + kx
                    nc.tensor.matmul(
                        ps[:], w_sb[:, (w_idx * 9 + t) * m:(w_idx * 9 + t + 1) * m],
                        src[:, off + nb: off + nb + nw],
                        start=(first and t == 0), stop=(last and t == 8))

            def zero_chunk_cols(st, nb, nw):
                """Zero the wrap-garbage columns (u=0 and u=65) inside this
                freshly written chunk of the padded state layout."""
                for rem in (0, Wp - 1):
                    q0 = ((nb - rem + Wp - 1) // Wp) * Wp + rem   # first >= nb
                    if q0 < nb + nw:
                        n_el = (nb + nw - 1 - q0) // Wp + 1
                        nc.vector.memset(
                            st[:, INT0 + q0: INT0 + q0 + n_el * Wp: Wp], 0.0)

            cc_ins = [dp.tile([128, NINT], F16, tag=f"ccin{s}", name=f"ccin{s}")
                      for s in range(STEPS)]
            cc_outs = [dp.tile([128, NINT], F16, tag=f"ccout{s}", name=f"ccout{s}")
                       for s in range(STEPS)]

            def accum_lsum(gp_or_pp, j, sts_now, nb, nw, cc_in):
                """Fold object j's freshly evicted chunk into the running local
                sum; on the last object emit the fp16 payload chunk + DMA."""
                acc = lsum_sb[:, nb:nb + nw]
                s = sts_now[j][:, INT0 + nb:INT0 + nb + nw]
                if j == 1:
                    nc.vector.tensor_add(acc, sts_now[0][:, INT0 + nb:INT0 + nb + nw], s)
                elif j == 2:
                    nc.vector.tensor_add(acc, acc, s)
                elif j == 3:
                    lt16 = gp_or_pp.tile([128, 512], F16, tag="lt16", bufs=2, name="lt16")
                    nc.vector.tensor_add(lt16[:, :nw], acc, s)
                    nc.sync.dma_start(out=cc_in[:, nb:nb + nw], in_=lt16[:, :nw])

            cur = [sts[0], sts[1], sts[2], sts[3]]
            spare = sts[4]

            for _rep in range(repeat):
                with tc.tile_pool(name="encpool", bufs=1) as ep:
                    encw_sb = ep.tile([128, 2 * 9 * 128], F16, tag="encw")
                    maskw_sb = ep.tile([128, 128], F16, tag="maskw")
                    mask_sb = ep.tile([128, NINT], F16, tag="maskcols")
                    # slice boundaries follow the first chunks' read windows
                    wcuts = [0, 128, 416, 704, 1152, 1280, 1568, 1856, 2304]
                    cuts = [0, 615, 1815, 3015, EXT]
                    fslices = [(k, a, b) for a, b in zip(cuts, cuts[1:])
                               for k in range(2)]
                    for i in range(8):
                        nc.sync.dma_start(out=encw_sb[:, wcuts[i]:wcuts[i + 1]],
                                          in_=encw_ap[:, wcuts[i]:wcuts[i + 1]])
                        k, fa, fb = fslices[i]
                        nc.sync.dma_start(out=feats_sb[k][:, fa:fb],
                                          in_=feats_ap[k, :, fa:fb])
                    nc.sync.dma_start(out=maskw_sb[:], in_=maskw_ap[:])
                    for j in range(4):
                        nc.sync.dma_start(out=mask_sb[32 * j:32 * j + 9, :],
                                          in_=mcols_ap[j])
                    nc.sync.dma_start(out=encb_sb[:], in_=encb_ap[:])
                    if _rep == 0:
                        nc.sync.dma_start(out=ident_sb[:], in_=ident_ap[:])
                    if _rep == 0:
                        nc.sync.dma_start(out=gcnb_sb[:], in_=gcnb_ap[:])
                        nc.sync.dma_start(out=rob_sb[:], in_=rob_ap[:])
                        nc.sync.dma_start(out=gcnw12_sb[:], in_=gcnw12_ap[:])
                        nc.sync.dma_start(out=gcnw2_sb[:], in_=gcnw2_ap[:])
                        nc.sync.dma_start(out=row_sb[:], in_=row_ap[:])

                    # ================= ENC =================
                    # single chunk loop: encF conv, then per-object mask+identity
                    # so evictions drain while later chunks' matmuls stream
                    cc_in0 = cc_ins[0]
                    for nb, nw in CHUNKS:
                        ps = psp.tile([128, nw], F32, tag="cps")
                        for kt in range(2):
                            conv_mms(ps, encw_sb, kt, feats_sb[kt], nb, nw,
                                     first=(kt == 0), last=(kt == 1))
                        nc.scalar.activation(shared_sb[:, nb:nb + nw], ps[:], AF.Copy)
                        pss = [psp.tile([128, nw], F32, tag="cps", name=f"mps{j}")
                               for j in range(4)]
                        for j in range(4):
                            nc.tensor.matmul(pss[j][:], maskw_sb[32 * j:32 * j + 9, :],
                                             mask_sb[32 * j:32 * j + 9, nb:nb + nw],
                                             start=True, stop=False,
                                             tile_position=(32 * j, 0))
                        for j in range(4):
                            nc.tensor.matmul(pss[j][:], ident_sb[:],
                                             shared_sb[:, nb:nb + nw],
                                             start=False, stop=True)
                        for j in range(4):
                            st = cur[j]
                            nc.scalar.activation(st[:, INT0 + nb:INT0 + nb + nw],
                                                 pss[j][:], AF.Relu, bias=encb_sb[:])
                            zero_chunk_cols(st, nb, nw)
                            accum_lsum(ep, j, cur, nb, nw, cc_in0)

                # ================= GCN x2 =================
                with tc.tile_pool(name="gcnpool", bufs=1) as gp:
                    total_sb = gp.tile([128, EXT], F16, tag="total")
                    nc.vector.memset(total_sb[:, 0:INT0], 0.0)
                    nc.vector.memset(total_sb[:, INT0 + NINT:EXT], 0.0)
                    parks = [gp.tile([128, NINT], F32, tag=f"park{i}", name=f"park{i}")
                             for i in range(4)]
                    for step in range(STEPS):
                        # payload was produced during the previous phase's evictions
                        cc_in = cc_ins[step]
                        cc_out = cc_outs[step]
                        nc.gpsimd.collective_compute(
                            "AllReduce", mybir.AluOpType.add,
                            replica_groups=[[0, 1], [2, 3], [4, 5], [6, 7]],
                            ins=[cc_in.opt()], outs=[cc_out.opt()])
                        for nb, nw in CHUNKS:
                            nc.sync.dma_start(out=total_sb[:, INT0 + nb:INT0 + nb + nw],
                                              in_=cc_out[:, nb:nb + nw])

                        # park objects 0,1 while the AllReduce flies
                        for j in range(2):
                            for nb, nw in CHUNKS:
                                ps = psp.tile([128, nw], F32, tag="cps")
                                conv_mms(ps, gcnw12_sb, 0, cur[j], nb, nw, True, True)
                                nc.scalar.activation(parks[j][:, nb:nb + nw], ps[:],
                                                     AF.Copy)
                        # conv(total, w2) mid-phase: collective has landed by now,
                        # so the finalize evictions overlap the remaining parks
                        for nb, nw in CHUNKS:
                            ps = psp.tile([128, nw], F32, tag="cps")
                            conv_mms(ps, gcnw2_sb, 0, total_sb, nb, nw, True, True)
                            nc.scalar.activation(shared_sb[:, nb:nb + nw], ps[:], AF.Copy)
                        for j in (2, 3):
                            for nb, nw in CHUNKS:
                                ps = psp.tile([128, nw], F32, tag="cps")
                                conv_mms(ps, gcnw12_sb, 0, cur[j], nb, nw, True, True)
                                nc.scalar.activation(parks[j][:, nb:nb + nw], ps[:],
                                                     AF.Copy)
                        # finalize: states_new = relu(park + shared + bias)
                        dsts = [spare, cur[0], cur[1], cur[2]]
                        cc_in_next = cc_ins[1] if step == 0 else None
                        order = ([(j, nb, nw) for nb, nw in CHUNKS for j in (0, 1)]
                                 + [(j, nb, nw) for j in (2, 3) for nb, nw in CHUNKS])
                        for j, nb, nw in order:
                            dst = dsts[j]
                            if True:
                                nc.vector.tensor_add(parks[j][:, nb:nb + nw],
                                                     parks[j][:, nb:nb + nw],
                                                     shared_sb[:, nb:nb + nw])
                                nc.scalar.activation(dst[:, INT0 + nb:INT0 + nb + nw],
                                                     parks[j][:, nb:nb + nw],
                                                     AF.Relu, bias=gcnb_sb[:])
                                zero_chunk_cols(dst, nb, nw)
                                if step == 0:
                                    accum_lsum(gp, j, dsts, nb, nw, cc_in_next)
                        new_spare = cur[3]
                        cur = [dsts[0], dsts[1], dsts[2], dsts[3]]
                        spare = new_spare

                # ================= READOUT =================
                # M=4: 54 (ktile, tap) accumulating matmuls split over 4 PE
                # column strips, issued round-robin for strip concurrency.
                with tc.tile_pool(name="ropool", bufs=1) as rp:
                    out_sb = rp.tile([4, NINT], F32, tag="outsb")
                    strips = [
                        [(4, t) for t in range(9)] + [(2, t) for t in range(5)],
                        [(5, t) for t in range(9)] + [(2, t) for t in range(5, 9)]
                        + [(3, 0)],
                        [(0, t) for t in range(4)] + [(1, t) for t in range(5)]
                        + [(3, t) for t in range(1, 5)],
                        [(0, t) for t in range(4, 9)] + [(1, t) for t in range(5, 9)]
                        + [(3, t) for t in range(5, 9)],
                    ]
                    for nb, nw in CHUNKS:
                        pss = [psp.tile([128, nw], F32, tag="cps", name=f"rops{g}")
                               for g in range(4)]
                        for i in range(14):
                            for g, chain in enumerate(strips):
                                if i >= len(chain):
                                    continue
                                k, t = chain[i]
                                src = cur[k] if k < 4 else feats_sb[k - 4]
                                ky, kx = TAPS[t]
                                off = ky * Wp + kx
                                nc.tensor.matmul(
                                    pss[g][32 * g:32 * g + 4, :],
                                    row_sb[:, (k * 9 + t) * 4:(k * 9 + t + 1) * 4],
                                    src[:, off + nb: off + nb + nw],
                                    start=(i == 0), stop=(i == len(chain) - 1),
                                    tile_position=(0, 32 * g))
                        o = out_sb[:, nb:nb + nw]
                        nc.vector.tensor_copy(o, pss[0][0:4, :])
                        nc.vector.tensor_add(o, o, pss[1][32:36, :])
                        nc.vector.tensor_add(o, o, pss[2][64:68, :])
                        nc.vector.tensor_add(o, o, pss[3][96:100, :])
                        nc.scalar.activation(o, o, AF.Sigmoid, bias=rob_sb[:])
                    ov = out_ap.rearrange("o (y x) -> o y x", x=W)
                    iv = out_sb[:].rearrange("o (y x) -> o y x", x=Wp)[:, :, 1:1 + W]
                    for r in range(0, 64, 8):
                        nc.sync.dma_start(out=ov[:, r:r + 8], in_=iv[:, r:r + 8])

    nc.compile()
    return nc


def _host_prep(inputs):
    """Per-core input maps: shard + pad + im2col + weight lhsT layouts."""
    feats = np.asarray(inputs["batch_node_feats"], np.float32)
    masks = np.asarray(inputs["batch_previous_masks"], np.float32)
    enc_w = np.asarray(inputs["enc_w"], np.float32)
    enc_b = np.asarray(inputs["enc_b"], np.float32)
    gcn_w = np.asarray(inputs["gcn_w"], np.float32)
    gcn_b = np.asarray(inputs["gcn_b"], np.float32)
    ro_w = np.asarray(inputs["ro_w"], np.float32)
    ro_b = np.asarray(inputs["ro_b"], np.float32)

    # ---- weights (shared across cores) ----
    # enc feats part: [2, 9, 128cin, 128cout] lhsT per (ktile, tap)
    # [128cin_part, ktile, tap, cout] contiguous per partition
    encw = enc_w[:, :C].transpose(2, 3, 1, 0).reshape(9, 2, 128, HID) \
        .transpose(2, 1, 0, 3).reshape(128, 2 * 9 * HID).copy()
    # enc mask channel: K=9 lhsT replicated at partitions {0,32,64,96}
    mvec = enc_w[:, C].transpose(1, 2, 0).reshape(9, HID)  # [tap, cout]
    maskw = np.zeros((128, 128), np.float32)
    for j in range(4):
        maskw[32 * j:32 * j + 9] = mvec
    w1 = gcn_w[:, :HID]
    w2 = gcn_w[:, HID:]
    gcnw12 = (w1 - w2).transpose(2, 3, 1, 0).reshape(9, 128, 128) \
        .transpose(1, 0, 2).reshape(128, 9 * 128).copy()
    gcnw2 = w2.transpose(2, 3, 1, 0).reshape(9, 128, 128) \
        .transpose(1, 0, 2).reshape(128, 9 * 128).copy()
    # readout: [6, 9, 128, 4]
    row = np.zeros((6, 9, 128, 4), np.float32)
    rs = ro_w[0, C:].transpose(1, 2, 0).reshape(9, HID)   # states part [tap, cin]
    for k in range(4):
        row[k, :, :, k] = rs
    for k, sl in ((4, ro_w[0, :128]), (5, ro_w[0, 128:256])):
        row[k] = sl.transpose(1, 2, 0).reshape(9, 128)[:, :, None]
    encb = enc_b.reshape(128, 1).astype(np.float32)
    gcnb = gcn_b.reshape(128, 1).astype(np.float32)
    rob = np.broadcast_to(ro_b.reshape(1, 1), (4, 1)).astype(np.float32).copy()

    in_maps = []
    for c in range(N_CORES):
        s, half = c // 2, c % 2
        # feats: pad to [C, 66, 66], flat ext [C, 4358] at offset 1
        fp = np.zeros((C, H + 2, Wp), np.float32)
        fp[:, 1:H + 1, 1:W + 1] = feats[s]
        fe = np.zeros((C, EXT), np.float32)
        fe[:, 1:1 + PADF] = fp.reshape(C, PADF)
        # masks im2col: [4, 9, NINT]
        mc = np.zeros((4, 9, NINT), np.float32)
        for j in range(4):
            mp = np.zeros((H + 2, Wp), np.float32)
            mp[1:H + 1, 1:W + 1] = masks[s, 4 * half + j]
            mf = np.zeros(EXT, np.float32)
            mf[1:1 + PADF] = mp.reshape(PADF)
            for t, (ky, kx) in enumerate(TAPS):
                off = ky * Wp + kx
                mc[j, t] = mf[off:off + NINT]
        in_maps.append({
            "feats": fe.reshape(2, 128, EXT).astype(np.float16),
            "mcols": mc.astype(np.float16),
            "encw": encw.astype(np.float16), "maskw": maskw.astype(np.float16),
            "gcnw12": gcnw12.astype(np.float16), "gcnw2": gcnw2.astype(np.float16),
            "row": row.transpose(2, 0, 1, 3).reshape(128, 6 * 9 * 4).astype(np.float16),
            "ident": np.eye(128, dtype=np.float16),
            "encb": encb, "gcnb": gcnb, "rob": rob,
        })
    return in_maps


def _run(inputs, repeat=1):
    from concourse.bass_utils import run_bass_kernel_spmd
    if repeat not in _PROG_CACHE:
        _PROG_CACHE[repeat] = _build_program(repeat)
    nc = _PROG_CACHE[repeat]
    in_maps = _host_prep(inputs)
    r = run_bass_kernel_spmd(nc, in_maps, list(range(N_CORES)))
    out = np.zeros((B, O, H, W), np.float32)
    for c in range(N_CORES):
        s, half = c // 2, c % 2
        out[s, 4 * half:4 * half + 4] = r.results[c]["out"].reshape(4, H, W)
    return out


def kernel(**inputs) -> np.ndarray:
    return _run(inputs, repeat=1)



# revision 6
# speedup vs baseline: 1.0632x; 1.0632x over previous
"""Bass/Trainium2 kernel for nn_GCNNTemporal (GNN message passing over object masks).

Reference computation (B=4 samples, O=8 objects, C=256, HID=128, H=W=64):
  states = relu(conv3x3(concat(feats, mask_o)))          per (sample, object)
  2x:  states_o = relu(conv3x3(concat(states_o, sum_{j!=o} states_j)))
  out_o = sigmoid(conv3x3(concat(feats, states_o)))

Sharding: 2 cores per sample, 4 objects per core. Neighbor sums need all 8
objects; the shared conv(total, w2) is deduplicated across the core pair via
ReduceScatter (halo-extended local sums) -> conv own half -> AllGather.

Layout: 65-column rows (shared single pad column between rows: position
(y, 64) is the zero that serves as both right-pad of row y and left-pad of
row y+1). Conv3x3 SAME = 9 shifted matmuls accumulating in PSUM; shifts are
pure AP offsets. Finalize ops write through strided APs that skip the pad
column, so pad zeros are never clobbered (no per-chunk re-zeroing).

Algebra:
  - enc: conv(feats) shared by the 4 objects (encF psum -> fp16 shared);
    per-object K=9 im2col mask matmul; finalize st = relu(maskps + shared + b).
  - gcn: agg_o = total - states_o  =>  st_o' = relu(conv(st_o, w1-w2)
    + conv(total, w2) + b); conv(total, w2) computed on HALF the rows per
    core (RS gives each core its halo-extended half of total), AllGather
    rebuilds the full shared tensor.
  - readout (Cout=1): 4 objects + 2 feats k-tile groups stacked into
    block-(diagonal/broadcast) weights (M=4), 4 concurrent PE column strips.
"""
import sys
sys.path.insert(0, '/opt/trn_rl_repo')
import numpy as np

B, O, C, HID, H, W = 4, 8, 256, 128, 64, 64
STEPS = 2
N_CORES = 8

Wp = W + 1                  # 65: shared pad column layout
NINT = H * Wp               # 4160 interior flat size
GUARD = Wp + 1              # 66 zero guard elements on each side
EXT = GUARD + NINT + GUARD  # 4292
INT0 = GUARD                # interior offset
# full-area chunks: 9 x 7 rows + 1 x 1 row (psum bank limit: 7*65*4B < 2KB)
CHUNKS = [(i * 7 * Wp, 7 * Wp) for i in range(9)] + [(63 * Wp, Wp)]
# half-area chunks for the shared conv (32 rows)
HCHUNKS = [(i * 7 * Wp, 7 * Wp) for i in range(4)] + [(28 * Wp, 4 * Wp)]
TAPS = [(ky, kx) for ky in range(3) for kx in range(3)]
HROWS = 34                  # halo-extended half: 1 + 32 + 1 rows
HCOLS = HROWS * Wp          # 2210 payload cols per RS chunk
TOPN = 33 * Wp              # rows 0..32 slice size (2145)

_PROG_CACHE = {}


def _build_program(repeat=1):
    import concourse.tile as tile
    from concourse import bacc, mybir

    AF = mybir.ActivationFunctionType
    F32 = mybir.dt.float32
    F16 = mybir.dt.float16

    nc = bacc.Bacc("TRN2", target_bir_lowering=False, debug=False,
                   num_devices=N_CORES)

    # ---- DRAM I/O ----
    feats_ap = nc.dram_tensor("feats", [2, 128, EXT], F16, kind="ExternalInput").ap()
    mcols_ap = nc.dram_tensor("mcols", [4, 9, NINT], F16, kind="ExternalInput").ap()
    encw_ap = nc.dram_tensor("encw", [128, 2 * 9 * 128], F16, kind="ExternalInput").ap()
    maskw_ap = nc.dram_tensor("maskw", [128, 128], F16, kind="ExternalInput").ap()
    gcnw12_ap = nc.dram_tensor("gcnw12", [128, 9 * 128], F16, kind="ExternalInput").ap()
    gcnw2_ap = nc.dram_tensor("gcnw2", [128, 9 * 128], F16, kind="ExternalInput").ap()
    row_ap = nc.dram_tensor("row", [128, 6 * 9 * 4], F16, kind="ExternalInput").ap()
    encb_ap = nc.dram_tensor("encb", [128, 1], F32, kind="ExternalInput").ap()
    gcnb_ap = nc.dram_tensor("gcnb", [128, 1], F32, kind="ExternalInput").ap()
    rob_ap = nc.dram_tensor("rob", [4, 1], F32, kind="ExternalInput").ap()
    out_ap = nc.dram_tensor("out", [4, H * W], F32, kind="ExternalOutput").ap()

    def skip(ap_2d, nb, nw):
        """Strided view of interior chunk [nb, nb+nw) of a [128, NINT]-region
        AP that skips the pad column (col 64 of each 65-wide row)."""
        nr = nw // Wp
        return ap_2d[:, nb:nb + nw].rearrange("p (r c) -> p r c", c=Wp)[:, :, 0:W]

    with tile.TileContext(nc) as tc:
        with tc.tile_pool(name="persist", bufs=1) as pp, \
             tc.tile_pool(name="psum", bufs=8, space="PSUM") as psp, \
             tc.tile_pool(name="dram", bufs=1, space="DRAM") as dp:

            # ---- persistent SBUF ----
            sts = [pp.tile([128, EXT], F16, tag=f"st{i}", name=f"st{i}")
                   for i in range(5)]
            feats_sb = pp.tile([128, 2, EXT], F16, tag="feats")
            shared_sb = pp.tile([128, NINT], F16, tag="shared")   # encF / gcn shared
            parks = [pp.tile([128, NINT], F16, tag=f"park{i}", name=f"park{i}")
                     for i in range(3)]
            lsum_sb = pp.tile([128, NINT], F16, tag="lsum")
            total_sb = pp.tile([128, HCOLS + 2], F16, tag="total")
            shhalf_sb = pp.tile([128, 32 * Wp], F16, tag="shhalf")
            mcols_sb = pp.tile([128, NINT], F16, tag="mcols")
            encw_sb = pp.tile([128, 2 * 9 * 128], F16, tag="encw")
            maskw_sb = pp.tile([128, 128], F16, tag="maskw")
            gcnw12_sb = pp.tile([128, 9 * 128], F16, tag="gw12")
            gcnw2_sb = pp.tile([128, 9 * 128], F16, tag="gw2")
            row_sb = pp.tile([128, 6 * 9 * 4], F16, tag="row")
            encb_sb = pp.tile([128, 1], F32, tag="encb")
            gcnb_sb = pp.tile([128, 1], F32, tag="gcnb")
            rob_sb = pp.tile([4, 1], F32, tag="rob")
            out_sb = pp.tile([4, NINT], F32, tag="outsb")
            zrow_sb = pp.tile([128, Wp], F16, tag="zrow")

            # ---- input DMAs: ENC-critical first ----
            wcuts = [0, 128, 416, 704, 1152, 1280, 1568, 1856, 2304]
            cuts = [0, 615, 1815, 3015, EXT]
            fslices = [(k, a, b) for a, b in zip(cuts, cuts[1:])
                       for k in range(2)]
            for i in range(8):
                nc.sync.dma_start(out=encw_sb[:, wcuts[i]:wcuts[i + 1]],
                                  in_=encw_ap[:, wcuts[i]:wcuts[i + 1]])
                k, fa, fb = fslices[i]
                nc.sync.dma_start(out=feats_sb[:, k, fa:fb],
                                  in_=feats_ap[k, :, fa:fb])
            nc.sync.dma_start(out=maskw_sb[:], in_=maskw_ap[:])
            for j in range(4):
                nc.sync.dma_start(out=mcols_sb[32 * j:32 * j + 9, :],
                                  in_=mcols_ap[j])
            nc.sync.dma_start(out=encb_sb[:], in_=encb_ap[:])
            nc.sync.dma_start(out=gcnb_sb[:], in_=gcnb_ap[:])
            nc.sync.dma_start(out=rob_sb[:], in_=rob_ap[:])
            nc.sync.dma_start(out=gcnw12_sb[:], in_=gcnw12_ap[:])
            nc.sync.dma_start(out=gcnw2_sb[:], in_=gcnw2_ap[:])
            nc.sync.dma_start(out=row_sb[:], in_=row_ap[:])

            # ---- one-time zeroing (pad never rewritten afterwards) ----
            nc.vector.memset(zrow_sb[:], 0.0)
            for t_ in sts:
                nc.vector.memset(t_[:, 0:INT0], 0.0)
                nc.vector.memset(t_[:, INT0 + NINT:EXT], 0.0)
                nc.vector.memset(t_[:, INT0 + W: INT0 + W + 63 * Wp + 1: Wp], 0.0)
            nc.vector.memset(total_sb[:, 0:1], 0.0)
            nc.vector.memset(total_sb[:, HCOLS + 1:HCOLS + 2], 0.0)

            # collective DRAM buffers (per step)
            cc_ins = [dp.tile([2, 128, HCOLS], F16, tag=f"ccin{s}", name=f"ccin{s}")
                      for s in range(STEPS)]
            cc_outs = [dp.tile([128, HCOLS], F16, tag=f"ccout{s}", name=f"ccout{s}")
                       for s in range(STEPS)]
            ag_ins = [dp.tile([128, 32 * Wp], F16, tag=f"agin{s}", name=f"agin{s}")
                      for s in range(STEPS)]
            ag_outs = [dp.tile([2, 128, 32 * Wp], F16, tag=f"agout{s}", name=f"agout{s}")
                       for s in range(STEPS)]
            # payload guard rows (zero): leading row of chunk0, trailing of chunk1
            for s in range(STEPS):
                nc.sync.dma_start(out=cc_ins[s][0, :, 0:Wp], in_=zrow_sb[:])
                nc.sync.dma_start(out=cc_ins[s][1, :, HCOLS - Wp:HCOLS], in_=zrow_sb[:])

            # tiny warm-up collective absorbs CC-stream init latency
            ccw_in = dp.tile([1, 1], F32, tag="ccwin")
            ccw_out = dp.tile([1, 1], F32, tag="ccwout")
            warm_sb = pp.tile([1, 1], F32, tag="warm")
            nc.vector.memset(warm_sb[:], 0.0)
            nc.sync.dma_start(out=ccw_in[:], in_=warm_sb[:])
            nc.gpsimd.collective_compute(
                "AllReduce", mybir.AluOpType.add,
                replica_groups=[[0, 1], [2, 3], [4, 5], [6, 7]],
                ins=[ccw_in.opt()], outs=[ccw_out.opt()])

            GROUPS = [[0, 1], [2, 3], [4, 5], [6, 7]]

            def conv_mms(ps, w_sb, w_idx, src, nb, nw, m=128):
                """9 accumulating tap matmuls into psum ps. src is a [128, *]
                AP whose offset 0 aligns with interior-66 (i.e. windows are
                src[:, nb+off : nb+off+nw])."""
                for t, (ky, kx) in enumerate(TAPS):
                    off = ky * Wp + kx
                    nc.tensor.matmul(
                        ps[:], w_sb[:, (w_idx * 9 + t) * m:(w_idx * 9 + t + 1) * m],
                        src[:, off + nb: off + nb + nw],
                        start=(t == 0), stop=(t == 8))

            def accum_lsum(j, sts_now, nb, nw, cc_in):
                """Fold object j's fresh chunk into the running local sum; on
                the last object DMA the halo-extended payload slices."""
                acc = lsum_sb[:, nb:nb + nw]
                s = sts_now[j][:, INT0 + nb:INT0 + nb + nw]
                if j == 1:
                    nc.vector.tensor_add(acc, sts_now[0][:, INT0 + nb:INT0 + nb + nw], s)
                elif j == 2:
                    nc.vector.tensor_add(acc, acc, s)
                elif j == 3:
                    nc.vector.tensor_add(acc, acc, s)
                    # payload: chunk0 = [zrow, rows 0..32], chunk1 = [rows 31..63, zrow]
                    lo, hi = nb, nb + nw
                    a, b_ = max(lo, 0), min(hi, TOPN)
                    if a < b_:
                        nc.sync.dma_start(out=cc_in[0, :, Wp + a:Wp + b_],
                                          in_=lsum_sb[:, a:b_])
                    a, b_ = max(lo, 31 * Wp), min(hi, NINT)
                    if a < b_:
                        nc.sync.dma_start(out=cc_in[1, :, a - 31 * Wp:b_ - 31 * Wp],
                                          in_=lsum_sb[:, a:b_])

            cur = [sts[0], sts[1], sts[2], sts[3]]
            spare = sts[4]

            for _rep in range(repeat):
                # ================= ENC =================
                with tc.tile_pool(name="encpool", bufs=1) as ep:
                    cc_in0 = cc_ins[0]
                    for nb, nw in CHUNKS:
                        nr = nw // Wp
                        ps = psp.tile([128, nw], F32, tag="cps")
                        for kt in range(2):
                            for t, (ky, kx) in enumerate(TAPS):
                                off = ky * Wp + kx
                                nc.tensor.matmul(
                                    ps[:], encw_sb[:, (kt * 9 + t) * 128:(kt * 9 + t + 1) * 128],
                                    feats_sb[:, kt, off + nb: off + nb + nw],
                                    start=(kt == 0 and t == 0), stop=(kt == 1 and t == 8))
                        nc.scalar.activation(shared_sb[:, nb:nb + nw], ps[:], AF.Copy)
                        pss = [psp.tile([128, nw], F32, tag="cps", name=f"mps{j}")
                               for j in range(4)]
                        for j in range(4):
                            nc.tensor.matmul(pss[j][:], maskw_sb[32 * j:32 * j + 9, :],
                                             mcols_sb[32 * j:32 * j + 9, nb:nb + nw],
                                             start=True, stop=True,
                                             tile_position=(32 * j, 0))
                        for j in range(4):
                            st = cur[j]
                            tmp = ep.tile([128, 7 * W], F16, tag="etmp", bufs=3,
                                          name="etmp")
                            tv = tmp[:, 0:nr * W].rearrange("p (r c) -> p r c", c=W)
                            nc.vector.tensor_add(
                                tv,
                                pss[j][:].rearrange("p (r c) -> p r c", c=Wp)[:, :, 0:W],
                                shared_sb[:, nb:nb + nw].rearrange("p (r c) -> p r c", c=Wp)[:, :, 0:W])
                            nc.scalar.activation(
                                skip(st[:, INT0:INT0 + NINT], nb, nw),
                                tv, AF.Relu, bias=encb_sb[:])
                            accum_lsum(j, cur, nb, nw, cc_in0)

                # ================= GCN x2 =================
                with tc.tile_pool(name="gcnpool", bufs=1) as gp:
                    for step in range(STEPS):
                        cc_in = cc_ins[step]
                        cc_out = cc_outs[step]
                        ag_in = ag_ins[step]
                        ag_out = ag_outs[step]
                        nc.gpsimd.collective_compute(
                            "ReduceScatter", mybir.AluOpType.add,
                            replica_groups=GROUPS,
                            ins=[cc_in.opt()], outs=[cc_out.opt()])
                        nc.sync.dma_start(out=total_sb[:, 1:1 + HCOLS], in_=cc_out[:])

                        # park objects 0,1 while the ReduceScatter flies
                        for j in range(2):
                            for nb, nw in CHUNKS:
                                ps = psp.tile([128, nw], F32, tag="cps")
                                conv_mms(ps, gcnw12_sb, 0, cur[j], nb, nw)
                                nc.scalar.activation(parks[j][:, nb:nb + nw], ps[:],
                                                     AF.Copy)
                        # conv(total_half, w2): windows live in total_sb at
                        # offset 1 (tap off 0..132 relative to out row base)
                        for nb, nw in HCHUNKS:
                            ps = psp.tile([128, nw], F32, tag="cps")
                            for t, (ky, kx) in enumerate(TAPS):
                                off = ky * Wp + kx
                                nc.tensor.matmul(
                                    ps[:], gcnw2_sb[:, t * 128:(t + 1) * 128],
                                    total_sb[:, off + nb: off + nb + nw],
                                    start=(t == 0), stop=(t == 8))
                            nc.scalar.activation(shhalf_sb[:, nb:nb + nw], ps[:],
                                                 AF.Copy)
                            nc.sync.dma_start(out=ag_in[:, nb:nb + nw],
                                              in_=shhalf_sb[:, nb:nb + nw])
                        nc.gpsimd.collective_compute(
                            "AllGather", mybir.AluOpType.bypass,
                            replica_groups=GROUPS,
                            ins=[ag_in.opt()], outs=[ag_out.opt()])
                        for h in range(2):
                            nc.sync.dma_start(
                                out=shared_sb[:, h * 32 * Wp:(h + 1) * 32 * Wp],
                                in_=ag_out[h])

                        # park object 2, then object 3 direct
                        for nb, nw in CHUNKS:
                            ps = psp.tile([128, nw], F32, tag="cps")
                            conv_mms(ps, gcnw12_sb, 0, cur[2], nb, nw)
                            nc.scalar.activation(parks[2][:, nb:nb + nw], ps[:],
                                                 AF.Copy)
                        dsts = [spare, cur[0], cur[1], cur[2]]
                        cc_in_next = cc_ins[1] if step == 0 else None

                        # finalize parked objects 0..2 chunk-wise
                        for nb, nw in CHUNKS:
                            nr = nw // Wp
                            for j in range(3):
                                pk = parks[j][:, nb:nb + nw].rearrange(
                                    "p (r c) -> p r c", c=Wp)[:, :, 0:W]
                                sh = shared_sb[:, nb:nb + nw].rearrange(
                                    "p (r c) -> p r c", c=Wp)[:, :, 0:W]
                                nc.vector.tensor_add(pk, pk, sh)
                                nc.scalar.activation(
                                    skip(dsts[j][:, INT0:INT0 + NINT], nb, nw),
                                    pk, AF.Relu, bias=gcnb_sb[:])
                        # object 3: conv + direct finalize (shared is ready)
                        for nb, nw in CHUNKS:
                            nr = nw // Wp
                            ps = psp.tile([128, nw], F32, tag="cps")
                            conv_mms(ps, gcnw12_sb, 0, cur[3], nb, nw)
                            tmp = gp.tile([128, 7 * W], F16, tag="gtmp", bufs=3,
                                          name="gtmp")
                            tv = tmp[:, 0:nr * W].rearrange("p (r c) -> p r c", c=W)
                            nc.vector.tensor_add(
                                tv,
                                ps[:].rearrange("p (r c) -> p r c", c=Wp)[:, :, 0:W],
                                shared_sb[:, nb:nb + nw].rearrange("p (r c) -> p r c", c=Wp)[:, :, 0:W])
                            nc.scalar.activation(
                                skip(dsts[3][:, INT0:INT0 + NINT], nb, nw),
                                tv, AF.Relu, bias=gcnb_sb[:])
                            if step == 0:
                                for j in range(4):
                                    accum_lsum(j, dsts, nb, nw, cc_in_next)
                        new_spare = cur[3]
                        cur = [dsts[0], dsts[1], dsts[2], dsts[3]]
                        spare = new_spare

                # ================= READOUT =================
                # M=4: 54 (ktile, tap) accumulating matmuls split over 4 PE
                # column strips, issued round-robin for strip concurrency.
                with tc.tile_pool(name="ropool", bufs=1) as rp:
                    strips = [
                        [(4, t) for t in range(9)] + [(2, t) for t in range(5)],
                        [(5, t) for t in range(9)] + [(2, t) for t in range(5, 9)]
                        + [(3, 0)],
                        [(0, t) for t in range(4)] + [(1, t) for t in range(5)]
                        + [(3, t) for t in range(1, 5)],
                        [(0, t) for t in range(4, 9)] + [(1, t) for t in range(5, 9)]
                        + [(3, t) for t in range(5, 9)],
                    ]
                    ov = out_ap.rearrange("o (y x) -> o y x", x=W)
                    for nb, nw in CHUNKS:
                        nr = nw // Wp
                        r0 = nb // Wp
                        pss = [psp.tile([128, nw], F32, tag="cps", name=f"rops{g}")
                               for g in range(4)]
                        for i in range(14):
                            for g, chain in enumerate(strips):
                                if i >= len(chain):
                                    continue
                                k, t = chain[i]
                                src = cur[k][:] if k < 4 \
                                    else feats_sb[:, k - 4, :]
                                ky, kx = TAPS[t]
                                off = ky * Wp + kx
                                nc.tensor.matmul(
                                    pss[g][32 * g:32 * g + 4, :],
                                    row_sb[:, (k * 9 + t) * 4:(k * 9 + t + 1) * 4],
                                    src[:, off + nb: off + nb + nw],
                                    start=(i == 0), stop=(i == len(chain) - 1),
                                    tile_position=(0, 32 * g))
                        o = out_sb[:, nb:nb + nw]
                        nc.vector.tensor_copy(o, pss[0][0:4, :])
                        nc.vector.tensor_add(o, o, pss[1][32:36, :])
                        nc.vector.tensor_add(o, o, pss[2][64:68, :])
                        nc.vector.tensor_add(o, o, pss[3][96:100, :])
                        ot = o.rearrange("p (r c) -> p r c", c=Wp)[:, :, 0:W]
                        nc.scalar.activation(ot, ot, AF.Sigmoid, bias=rob_sb[:])
                        nc.sync.dma_start(out=ov[:, r0:r0 + nr], in_=ot)

    nc.compile()
    return nc


def _host_prep(inputs):
    """Per-core input maps: shard + pad + im2col + weight lhsT layouts."""
    feats = np.asarray(inputs["batch_node_feats"], np.float32)
    masks = np.asarray(inputs["batch_previous_masks"], np.float32)
    enc_w = np.asarray(inputs["enc_w"], np.float32)
    enc_b = np.asarray(inputs["enc_b"], np.float32)
    gcn_w = np.asarray(inputs["gcn_w"], np.float32)
    gcn_b = np.asarray(inputs["gcn_b"], np.float32)
    ro_w = np.asarray(inputs["ro_w"], np.float32)
    ro_b = np.asarray(inputs["ro_b"], np.float32)

    # ---- weights (shared across cores) ----
    # enc feats part: [128cin_part, ktile, tap, cout] contiguous per partition
    encw = enc_w[:, :C].transpose(2, 3, 1, 0).reshape(9, 2, 128, HID) \
        .transpose(2, 1, 0, 3).reshape(128, 2 * 9 * HID).copy()
    # enc mask channel: K=9 lhsT replicated at partitions {0,32,64,96}
    mvec = enc_w[:, C].transpose(1, 2, 0).reshape(9, HID)  # [tap, cout]
    maskw = np.zeros((128, 128), np.float32)
    for j in range(4):
        maskw[32 * j:32 * j + 9] = mvec
    w1 = gcn_w[:, :HID]
    w2 = gcn_w[:, HID:]
    gcnw12 = (w1 - w2).transpose(2, 3, 1, 0).reshape(9, 128, 128) \
        .transpose(1, 0, 2).reshape(128, 9 * 128).copy()
    gcnw2 = w2.transpose(2, 3, 1, 0).reshape(9, 128, 128) \
        .transpose(1, 0, 2).reshape(128, 9 * 128).copy()
    # readout: [6, 9, 128, 4]
    row = np.zeros((6, 9, 128, 4), np.float32)
    rs = ro_w[0, C:].transpose(1, 2, 0).reshape(9, HID)   # states part [tap, cin]
    for k in range(4):
        row[k, :, :, k] = rs
    for k, sl in ((4, ro_w[0, :128]), (5, ro_w[0, 128:256])):
        row[k] = sl.transpose(1, 2, 0).reshape(9, 128)[:, :, None]
    encb = enc_b.reshape(128, 1).astype(np.float32)
    gcnb = gcn_b.reshape(128, 1).astype(np.float32)
    rob = np.broadcast_to(ro_b.reshape(1, 1), (4, 1)).astype(np.float32).copy()

    def flat65(img):
        """[*, 64, 64] -> [*, EXT] padded 65-col layout (zeros elsewhere)."""
        lead = img.shape[:-2]
        fe = np.zeros(lead + (EXT,), np.float32)
        v = fe[..., INT0:INT0 + NINT].reshape(lead + (H, Wp))
        v[..., :, 0:W] = img
        return fe

    in_maps = []
    for c in range(N_CORES):
        s, half = c // 2, c % 2
        fe = flat65(feats[s])                       # [C, EXT]
        mf = flat65(masks[s, 4 * half:4 * half + 4])  # [4, EXT]
        mc = np.zeros((4, 9, NINT), np.float32)
        for t, (ky, kx) in enumerate(TAPS):
            off = ky * Wp + kx
            mc[:, t] = mf[:, off:off + NINT]
        in_maps.append({
            "feats": fe.reshape(2, 128, EXT).astype(np.float16),
            "mcols": mc.astype(np.float16),
            "encw": encw.astype(np.float16), "maskw": maskw.astype(np.float16),
            "gcnw12": gcnw12.astype(np.float16), "gcnw2": gcnw2.astype(np.float16),
            "row": row.transpose(2, 0, 1, 3).reshape(128, 6 * 9 * 4).astype(np.float16),
            "encb": encb, "gcnb": gcnb, "rob": rob,
        })
    return in_maps


def _run(inputs, repeat=1):
    from concourse.bass_utils import run_bass_kernel_spmd
    if repeat not in _PROG_CACHE:
        _PROG_CACHE[repeat] = _build_program(repeat)
    nc = _PROG_CACHE[repeat]
    in_maps = _host_prep(inputs)
    r = run_bass_kernel_spmd(nc, in_maps, list(range(N_CORES)))
    out = np.zeros((B, O, H, W), np.float32)
    for c in range(N_CORES):
        s, half = c // 2, c % 2
        out[s, 4 * half:4 * half + 4] = r.results[c]["out"].reshape(4, H, W)
    return out


def kernel(**inputs) -> np.ndarray:
    return _run(inputs, repeat=1)
